# revision 7
# baseline (speedup 1.0000x reference)
"""KNN (k=16, 10 classes, distance-weighted vote) on 8 Trainium2 cores.

Strategy (index-parallel): shard the 100k train points across 8 cores
(12800 each, padded). Each core computes, for all 1024 queries, the
augmented-matmul score  s = x.t - 0.5*||t||^2  (rank-equivalent to -d^2/2)
in fp32 on the PE, then per 512-column PSUM chunk extracts the top-8
scores + indices with the DVE max8/max_index ops. The host merges the
8*25*8=1600 candidates per query, rescores a 32-wide short list exactly in
the reference's fp32 op order, applies a coverage certificate (a chunk
whose 8th-best could hide a top-16 element triggers an exact fallback for
that row), and computes the weighted vote.
"""
import numpy as np

N_NEIGHBORS = 16
N_CLASSES = 10
N_CORES = 8
B = 1024            # queries
DIM = 64
K = DIM + 1         # augmented contraction dim
N_TRAIN = 100000
SHARD = 12800       # padded per-core train points (25 * 512)
CHUNK = 512
# DVE spans: wide PSUM windows per max8/max_index pair (4 banks = 2048),
# remainder span of 512. Fewer DVE instructions -> less init/DRAIN overhead.
SPANS = [(i * 2048, 2048) for i in range(6)] + [(12288, 512)]
NSP = len(SPANS)                # 7 spans per query-block
QB = 128                        # queries per block
NQB = B // QB                   # 8 query blocks
CAND_PER_CORE = NSP * 8         # 56
PAD_SCORE = -2.0e9
TOPC = 32                       # short list width for exact rescoring

_cached = {}


def _build_nc():
    import concourse.mybir as mybir
    from concourse import bacc
    from concourse.tile import TileContext

    D = B + SHARD  # packed input columns: x_augT | t_augT shard
    nc = bacc.Bacc()
    inp = nc.dram_tensor("inp", [K, D], mybir.dt.float32, kind="ExternalInput")
    outv = nc.dram_tensor("outv", [B, CAND_PER_CORE], mybir.dt.float32,
                          kind="ExternalOutput")
    outi = nc.dram_tensor("outi", [B, CAND_PER_CORE], mybir.dt.uint32,
                          kind="ExternalOutput")
    with TileContext(nc) as tc:
        with tc.tile_pool(name="sb", bufs=1) as sb, \
             tc.tile_pool(name="res", bufs=1) as res, \
             tc.tile_pool(name="ps", bufs=2, space="PSUM") as ps:
            inp_sb = sb.tile([K, D], mybir.dt.float32)
            nc.sync.dma_start(inp_sb[:, :], inp[:, :])
            for qb in range(NQB):
                v8 = res.tile([QB, CAND_PER_CORE], mybir.dt.float32, tag=f"v{qb}")
                i8 = res.tile([QB, CAND_PER_CORE], mybir.dt.uint32, tag=f"i{qb}")
                lhsT = inp_sb[:, qb * QB:(qb + 1) * QB]
                for sp, (start, width) in enumerate(SPANS):
                    psum = ps.tile([QB, 2048], mybir.dt.float32, tag="psum")
                    for i in range(width // CHUNK):
                        col = start + i * CHUNK
                        nc.tensor.matmul(
                            psum[:, i * CHUNK:(i + 1) * CHUNK], lhsT=lhsT,
                            rhs=inp_sb[:, B + col:B + col + CHUNK],
                            start=True, stop=True)
                    nc.vector.max(out=v8[:, sp * 8:(sp + 1) * 8],
                                  in_=psum[:, :width])
                    nc.vector.max_index(out=i8[:, sp * 8:(sp + 1) * 8],
                                        in_max=v8[:, sp * 8:(sp + 1) * 8],
                                        in_values=psum[:, :width])
                nc.sync.dma_start(outv[qb * QB:(qb + 1) * QB, :], v8[:, :])
                nc.sync.dma_start(outi[qb * QB:(qb + 1) * QB, :], i8[:, :])
    nc.finalize()
    return nc


def _get_nc():
    if "nc" not in _cached:
        _cached["nc"] = _build_nc()
    return _cached["nc"]


def _ref_f32_dist(x_rows, tv, x2_rows, t2_sel):
    """Reference fp32 op order: d2 = (x2 - 2*xy) + t2; dist = sqrt(max(d2,0)).

    x_rows [Q, DIM] f32, tv [Q, C, DIM] f32, x2_rows [Q, 1] f32,
    t2_sel [Q, C] f32 -> dist [Q, C] f32.
    """
    xy = np.einsum("qd,qcd->qc", x_rows.astype(np.float64),
                   tv.astype(np.float64)).astype(np.float32)
    d2 = ((x2_rows - np.float32(2.0) * xy).astype(np.float32)
          + t2_sel).astype(np.float32)
    return np.sqrt(np.maximum(d2, np.float32(0.0))).astype(np.float32)


def _vote(dist, labels_sel):
    """dist [Q, 16] f32, labels_sel [Q, 16] int -> (argmax i32, proba f32)."""
    with np.errstate(divide="ignore"):
        w = (np.float32(1.0) / dist).astype(np.float32)
    inf_mask = np.isinf(w)
    inf_row = np.any(inf_mask, axis=1, keepdims=True)
    w = np.where(inf_row, inf_mask.astype(np.float32), w).astype(np.float32)
    proba = np.zeros((dist.shape[0], N_CLASSES), dtype=np.float32)
    rows = np.repeat(np.arange(dist.shape[0]), N_NEIGHBORS)
    np.add.at(proba, (rows, labels_sel.reshape(-1)), w.reshape(-1))
    s = proba.sum(axis=1, keepdims=True, dtype=np.float32)
    s = np.where(s == 0.0, np.float32(1.0), s)
    proba = (proba / s).astype(np.float32)
    return np.argmax(proba, axis=1).astype(np.int32), proba


def _exact_row(x_row, train_data, t2, x2_row):
    """Full exact fallback for one query row; returns (dist16, idx16)."""
    xy = (train_data.astype(np.float64) @ x_row.astype(np.float64)).astype(np.float32)
    d2 = ((np.float32(x2_row) - np.float32(2.0) * xy).astype(np.float32)
          + t2).astype(np.float32)
    dist = np.sqrt(np.maximum(d2, np.float32(0.0))).astype(np.float32)
    order = np.lexsort((np.arange(N_TRAIN), dist))[:N_NEIGHBORS]
    return dist[order], order


def kernel(x, train_data, train_labels, _trace=False):
    from concourse import bass_utils

    x = np.asarray(x, dtype=np.float32)
    train_data = np.asarray(train_data, dtype=np.float32)
    labels_np = np.asarray(train_labels)

    # ---- host prep: augmented transposed operands ----
    t2 = np.sum(train_data ** 2, axis=1, dtype=np.float32).astype(np.float32)
    x2 = np.sum(x ** 2, axis=1, dtype=np.float32).astype(np.float32)

    n_pad_total = N_CORES * SHARD
    t_augT = np.empty((K, n_pad_total), dtype=np.float32)
    t_augT[:DIM, :N_TRAIN] = train_data.T
    t_augT[:DIM, N_TRAIN:] = 0.0
    t_augT[DIM, :N_TRAIN] = np.float32(-0.5) * t2
    t_augT[DIM, N_TRAIN:] = PAD_SCORE
    x_augT = np.empty((K, B), dtype=np.float32)
    x_augT[:DIM, :] = x.T
    x_augT[DIM, :] = 1.0

    in_maps = []
    for c in range(N_CORES):
        shard = t_augT[:, c * SHARD:(c + 1) * SHARD]
        in_maps.append({"inp": np.ascontiguousarray(
            np.concatenate([x_augT, shard], axis=1))})

    nc = _get_nc()
    res = bass_utils.run_bass_kernel_spmd(
        nc, in_maps, core_ids=list(range(N_CORES)), trace=_trace)
    if _trace:
        _cached["last_exec_time_ns"] = res.exec_time_ns

    # ---- host merge ----
    vals = np.concatenate([np.asarray(r["outv"]) for r in res.results], axis=1)
    loc = np.concatenate([np.asarray(r["outi"]) for r in res.results],
                         axis=1).astype(np.int64)
    span_base = np.repeat(np.array([s for s, _ in SPANS]), 8)    # [56]
    base = np.concatenate([c * SHARD + span_base for c in range(N_CORES)])
    gidx = base[None, :] + loc                                   # [B, 448]

    # short list: top-TOPC by device score (desc), ties by asc global index
    part = np.argpartition(-vals, TOPC - 1, axis=1)[:, :TOPC]
    pv = np.take_along_axis(vals, part, axis=1)
    pg = np.take_along_axis(gidx, part, axis=1)

    # exact rescore in reference fp32 op order
    pg_c = np.minimum(pg, N_TRAIN - 1)       # pads never near the top; safety
    tv = train_data[pg_c]                                        # [B, TOPC, DIM]
    dist = _ref_f32_dist(x, tv, x2[:, None], t2[pg_c])           # [B, TOPC]
    # duplicates (max_index repeats an index on tied values): keep one copy
    order = np.lexsort((pg_c, dist), axis=1)                     # asc dist, idx
    dist_o = np.take_along_axis(dist, order, axis=1)
    g_o = np.take_along_axis(pg_c, order, axis=1)
    pv_o = np.take_along_axis(pv, order, axis=1)
    srt = np.argsort(g_o, axis=1, kind="stable")
    g_s = np.take_along_axis(g_o, srt, axis=1)
    dup_s = np.zeros_like(g_s, dtype=bool)
    dup_s[:, 1:] = g_s[:, 1:] == g_s[:, :-1]
    dup = np.zeros_like(dup_s)
    np.put_along_axis(dup, srt, dup_s, axis=1)
    dist_o = np.where(dup, np.float32(np.inf), dist_o)
    order2 = np.lexsort((g_o, dist_o), axis=1)[:, :N_NEIGHBORS]
    dist16 = np.take_along_axis(dist_o, order2, axis=1)          # [B, 16]
    idx16 = np.take_along_axis(g_o, order2, axis=1)              # [B, 16]

    # coverage certificate: a span reports only its top-8, so if a span's
    # 8th-best device score could still outrank the row's kept 16th, the
    # span might hide an unreported top-16 element -> exact fallback row.
    chunk8 = vals[:, 7::8]                                       # [B, NSP*8]
    v16_dev = np.sort(pv_o[:, :], axis=1)[:, -N_NEIGHBORS]       # 16th-largest
    suspicious = chunk8 >= (v16_dev[:, None] - np.float32(1e-3))
    strict_bad = suspicious.any(axis=1)
    for q in np.nonzero(strict_bad)[0]:
        d16, i16 = _exact_row(x[q], train_data, t2, x2[q])
        dist16[q] = d16
        idx16[q] = i16

    labels_sel = labels_np[idx16]
    lab, proba = _vote(dist16, labels_sel.astype(np.int64))
    return lab, proba
